# revision 28
# baseline (speedup 1.0000x reference)
"""GPT forward (V=32000,S=1024,D=768,L=6,H=12,FF=3072,B=4) on 8 trn2 NeuronCores.

Sharding: DP=4 core-pairs over batch B; TP=2 (Megatron) inside each pair:
  heads 6+6, FF 1536+1536, vocab 16000+16000 for the logits GEMM.
All GEMMs run in bf16 (PE 1 cycle/row vs 4 for fp32); PSUM accumulation is
fp32. Residual h is bf16. LayerNorm stats come from ones-matmul column sums
(sum and sum-of-squares), the per-token scale/shift are broadcast across
partitions with K=1 matmuls (float32r bitcast, full-rate), and applied as
two bf16 tensor_tensor ops.
Attention is transposed (sT[k,q]) and processed per (head, key-block):
one wide scores matmul per key block over all later queries, causal mask
added in PSUM via a constant-matrix matmul (-240 strictly-lower), a single
wide Exp per key block, and PV accumulated in PSUM across key blocks.
The softmax denominator rides along as a 65th ones-column of V; the final
normalize is reciprocal_approx_fast + K=1 broadcast matmul + one multiply.
TP AllReduce runs in bf16 with a Shared-scratchpad output buffer.
"""

import os
import sys

import numpy as np

for _p in ("/opt/trn_rl_repo",):
    if _p not in sys.path:
        sys.path.insert(0, _p)

V, S, D, L, H, FF = 32000, 1024, 768, 6, 12, 3072
B, T = 4, 1024
HD = D // H            # 64
NC_ = 8                # cores
TP = 2
NH = H // TP           # 6 local heads
DQK = NH * HD          # 384
FFSH = FF // TP        # 1536
VSH = V // TP          # 16000
P = 128
KD = D // P            # 6 k-chunks of d_model
KFF = FFSH // P        # 12
NT = T // P            # 8 token chunks
NB = 2                 # 512-wide token blocks
VBLK = 500             # vocab free-block
VN = VSH // VBLK       # 32
EPS = 1e-5
VW = 65                # v columns per head incl. ones column
MASKVAL = -240.0

_CACHE = {}


# --------------------------------------------------------------------------
# host-side input preparation (sharding + layout + LN folding)
# --------------------------------------------------------------------------

def _lhsT_layout(Wf, nm, nk):
    """Wf [nm*128 out, nk*128 in] -> [nm, 128(p=in%128), nk, 128(c=out%128)]
    so that sbuf tile[p, k*128+c] = Wf[m*128+c, k*128+p]."""
    return np.ascontiguousarray(
        Wf.reshape(nm, P, nk, P).transpose(0, 3, 2, 1)
    )


def _rhs_layout(Wf, nk, nblk):
    """Wf [nblk out, nk*128 in] -> [128(p), nk, nblk]: tile[p, k, c] = Wf[c, k*128+p]."""
    return np.ascontiguousarray(
        Wf.reshape(nblk, nk, P).transpose(2, 1, 0)
    )


def _bias_layout(b, nm):
    """b [nm*128] -> [128, nm]"""
    return np.ascontiguousarray(b.reshape(nm, P).T)


def prep_rank_weights(r, tok_emb, pos_emb, ln1_w, ln1_b, qkv_w, out_w,
                      ln2_w, ln2_b, up_w, down_w, lnf_w, lnf_b):
    """Weights depend only on the TP rank r (shared across the 4 DP pairs)."""
    import ml_dtypes
    bf16 = ml_dtypes.bfloat16
    f32 = np.float32

    inp = {}
    wqk = np.empty((L, KD, P, KD, P), bf16)
    bqk = np.empty((L, P, KD), f32)
    wv = np.empty((L, P, KD, DQK), bf16)
    wo = np.empty((L, KD, 64, NH, P), bf16)
    bo_row = np.empty((L, KD, P), bf16)
    wup = np.empty((L, KFF, P, KD, P), bf16)
    bup = np.empty((L, P, KFF), f32)
    wdn = np.empty((L, KD, P, KFF, P), bf16)

    hsel = slice(r * DQK, (r + 1) * DQK)
    for l in range(L):
        q_raw = qkv_w[l, 0 * D + r * DQK: 0 * D + (r + 1) * DQK]   # [384, 768]
        k_raw = qkv_w[l, 1 * D + r * DQK: 1 * D + (r + 1) * DQK]
        v_raw = qkv_w[l, 2 * D + r * DQK: 2 * D + (r + 1) * DQK]
        qk_raw = np.concatenate([q_raw, k_raw], 0)                 # [768, 768]
        wqk[l] = _lhsT_layout(qk_raw * ln1_w[l][None, :], KD, KD)
        bqk[l] = _bias_layout(qk_raw @ ln1_b[l], KD)
        wv[l] = _rhs_layout(v_raw * ln1_w[l][None, :], KD, DQK)
        bv = v_raw @ ln1_b[l]                                      # [384]
        wo_raw = out_w[l][:, hsel]                                 # [768, 384]
        # halved: the pair AllReduce of (h/2 + Wo/2 y + bo/2) returns h_new/2
        # [m, p(64), hh, c]: tile[p, hh*128+c] = wo_raw[m*128+c, hh*64+p]
        wo[l] = np.ascontiguousarray(
            (0.5 * wo_raw).reshape(KD, P, NH, 64).transpose(0, 3, 2, 1))
        bo_row[l] = (0.5 * (wo_raw @ bv)).reshape(KD, P)
        up_raw = up_w[l, r * FFSH:(r + 1) * FFSH]                  # [1536, 768]
        wup[l] = _lhsT_layout(up_raw * ln2_w[l][None, :], KFF, KD)
        bup[l] = _bias_layout(up_raw @ ln2_b[l], KFF)
        dn_raw = down_w[l][:, r * FFSH:(r + 1) * FFSH]             # [768, 1536]
        wdn[l] = _lhsT_layout(0.5 * dn_raw, KD, KFF)

    inp["wqk"], inp["bqk"], inp["wv"] = wqk, bqk, wv
    inp["wo"], inp["bo_row"] = wo, bo_row
    inp["wup"], inp["bup"], inp["wdn"] = wup, bup, wdn

    te = tok_emb[r * VSH:(r + 1) * VSH].astype(f32) * lnf_w[None, :].astype(f32)
    # [VN, 128, KD, VBLK]: tile[n, p, k, c] = te[n*VBLK + c, k*128 + p]
    inp["temb"] = np.ascontiguousarray(
        te.reshape(VN, VBLK, KD, P).transpose(0, 3, 2, 1)
    ).astype(bf16)
    return inp


def prep_all_inputs(**inputs):
    import ml_dtypes
    bf16 = ml_dtypes.bfloat16
    f32 = np.float32
    args = {k: np.asarray(v) for k, v in inputs.items()}
    for k in args:
        if args[k].dtype in (np.float64,):
            args[k] = args[k].astype(f32)
    idx = args.pop("idx")
    rank_w = [prep_rank_weights(r, **args) for r in range(TP)]

    in_maps = []
    for c in range(NC_):
        b, r = c // TP, c % TP
        inp = dict(rank_w[r])
        h0 = 0.5 * (args["tok_emb"][idx[b]] + args["pos_emb"][:T]).astype(f32).T
        inp["h0"] = np.ascontiguousarray(
            h0.reshape(KD, P, T).transpose(1, 0, 2)).astype(bf16)
        in_maps.append(inp)
    return in_maps


# --------------------------------------------------------------------------
# bass program
# --------------------------------------------------------------------------

def build_program(debug=False):
    import concourse.bass as bass
    import concourse.mybir as mybir
    import concourse.tile as tile
    from concourse import bacc
    from concourse.masks import make_upper_triangular, make_identity
    from contextlib import ExitStack

    f32 = mybir.dt.float32
    f32r = mybir.dt.float32r
    bf16 = mybir.dt.bfloat16
    AF = mybir.ActivationFunctionType
    Alu = mybir.AluOpType

    nc = bacc.Bacc(None, target_bir_lowering=False, debug=False, num_devices=NC_)

    din = {}
    din["h0"] = nc.dram_tensor("h0", [P, KD, T], bf16, kind="ExternalInput")
    din["wqk"] = nc.dram_tensor("wqk", [L, KD, P, KD, P], bf16, kind="ExternalInput")
    din["bqk"] = nc.dram_tensor("bqk", [L, P, KD], f32, kind="ExternalInput")
    din["wv"] = nc.dram_tensor("wv", [L, P, KD, DQK], bf16, kind="ExternalInput")
    din["wo"] = nc.dram_tensor("wo", [L, KD, 64, NH, P], bf16, kind="ExternalInput")
    din["bo_row"] = nc.dram_tensor("bo_row", [L, KD, P], bf16, kind="ExternalInput")
    din["wup"] = nc.dram_tensor("wup", [L, KFF, P, KD, P], bf16, kind="ExternalInput")
    din["bup"] = nc.dram_tensor("bup", [L, P, KFF], f32, kind="ExternalInput")
    din["wdn"] = nc.dram_tensor("wdn", [L, KD, P, KFF, P], bf16, kind="ExternalInput")
    din["temb"] = nc.dram_tensor("temb", [VN, P, KD, VBLK], bf16, kind="ExternalInput")
    dout = nc.dram_tensor("logits", [T, VSH], bf16, kind="ExternalOutput")
    ddbg = {}
    if debug:
        for nm, shp in (("dbg_xln", [P, KD * T]), ("dbg_qk", [P, KD * T]),
                        ("dbg_vt", [P, NT * NH * VW]),
                        ("dbg_y", [64, NH * T]),
                        ("dbg_h1", [P, KD * T]), ("dbg_h2", [P, KD * T]),
                        ("dbg_st", [P, T]), ("dbg_pt", [P, T]),
                        ("dbg_av", [VW, T])):
            ddbg[nm] = nc.dram_tensor(nm, shp, bf16, kind="ExternalOutput")
        ddbg["dbg_rs"] = nc.dram_tensor("dbg_rs", [1, T], f32, kind="ExternalOutput")
        ddbg["dbg_rbc"] = nc.dram_tensor("dbg_rbc", [64, T], f32, kind="ExternalOutput")

    groups = [[2 * i, 2 * i + 1] for i in range(NC_ // TP)]

    with tile.TileContext(nc) as tc:
        pers = ExitStack()

        const = pers.enter_context(tc.tile_pool(name="const", bufs=1))
        ones_b = const.tile([P, 512], bf16)
        nc.vector.memset(ones_b[:], 1.0)
        maskT = const.tile([P, P], bf16)
        make_upper_triangular(nc, maskT[:], val=MASKVAL, diag=False)
        ident = const.tile([P, P], bf16)
        make_identity(nc, ident[:])
        eps_t = const.tile([1, 1], f32)
        nc.vector.memset(eps_t[:], EPS)

        hp = pers.enter_context(tc.tile_pool(name="hp", bufs=1))
        h = hp.tile([P, KD * T], bf16)          # resident value is h/2
        xp = pers.enter_context(tc.tile_pool(name="xp", bufs=1))
        xln = xp.tile([P, KD * T], bf16)
        qkp = pers.enter_context(tc.tile_pool(name="qkp", bufs=1))
        qk = qkp.tile([P, KD * T], bf16)
        vtp = pers.enter_context(tc.tile_pool(name="vtp", bufs=1))
        vT = vtp.tile([P, NT * NH * VW], bf16)
        yp = pers.enter_context(tc.tile_pool(name="yp", bufs=1))
        y = yp.tile([64, NH * T], bf16)
        gp = pers.enter_context(tc.tile_pool(name="gp", bufs=1))
        g = gp.tile([P, KFF * T], bf16)
        prtp = pers.enter_context(tc.tile_pool(name="prtp", bufs=1))
        partial = prtp.tile([P, KD * T], bf16)
        abcp = pers.enter_context(tc.tile_pool(name="abcp", bufs=1))
        abc = abcp.tile([P, 2 * T], bf16)

        ptp = pers.enter_context(tc.tile_pool(name="ptp", bufs=4))
        sqp = pers.enter_context(tc.tile_pool(name="sqp", bufs=2))
        rbcp = pers.enter_context(tc.tile_pool(name="rbcp", bufs=2))
        lnp = pers.enter_context(tc.tile_pool(name="lnp", bufs=1))
        rsp = pers.enter_context(tc.tile_pool(name="rsp", bufs=2))

        # 4 + 4 PSUM banks: every tile is <= [128, 512] fp32 (one bank)
        ps_sm = pers.enter_context(tc.tile_pool(name="ps_sm", bufs=4, space="PSUM"))
        ps_av = pers.enter_context(tc.tile_pool(name="ps_av", bufs=4, space="PSUM"))

        dram = pers.enter_context(tc.tile_pool(name="dram", bufs=4, space="DRAM"))
        dbp = pers.enter_context(tc.tile_pool(name="dbp", bufs=2)) if debug else None

        wqkp = pers.enter_context(tc.tile_pool(name="wqkp", bufs=3))
        wvp = pers.enter_context(tc.tile_pool(name="wvp", bufs=1))
        wop = pers.enter_context(tc.tile_pool(name="wop", bufs=2))
        wupp = pers.enter_context(tc.tile_pool(name="wupp", bufs=3))
        wdnp = pers.enter_context(tc.tile_pool(name="wdnp", bufs=2))
        tep = pers.enter_context(tc.tile_pool(name="tep", bufs=3))
        osp = pers.enter_context(tc.tile_pool(name="osp", bufs=2))
        bp = pers.enter_context(tc.tile_pool(name="bp", bufs=3))

        nc.sync.dma_start(out=h[:].rearrange("p (k t) -> p k t", k=KD),
                          in_=din["h0"][:])
        # ones column (index 64) of every per-head V block, for denominator
        vT_4d = vT[:].rearrange("p (t h c) -> p t h c", t=NT, h=NH)
        nc.vector.memset(vT_4d[:, :, :, 64:65], 1.0)

        def dbg_dump(name, ap):
            if debug:
                nc.sync.dma_start(out=ddbg[name][:, :], in_=ap)

        def layernorm(src, dst):
            """Per-token LN of 2*src (src holds h/2) -> dst, [P, KD*T] bf16."""
            s1 = [ps_av.tile([1, 512], f32, tag="av", name=f"s1_{i}")
                  for i in range(NB)]
            s2 = [ps_av.tile([1, 512], f32, tag="av", name=f"s2_{i}")
                  for i in range(NB)]
            for k in range(KD):
                sq = sqp.tile([P, T], bf16, tag="sq")
                nc.scalar.square(sq[:], src[:, k * T:(k + 1) * T])
                for nb in range(NB):
                    nc.tensor.matmul(s1[nb][0:1, :], ones_b[:, 0:1],
                                     src[:, k * T + nb * 512: k * T + (nb + 1) * 512],
                                     start=(k == 0), stop=(k == KD - 1),
                                     skip_group_check=True)
                    nc.tensor.matmul(s2[nb][0:1, :], ones_b[:, 0:1],
                                     sq[:, nb * 512:(nb + 1) * 512],
                                     start=(k == 0), stop=(k == KD - 1),
                                     skip_group_check=True)
            lnt = lnp.tile([1, 3 * T], f32, tag="lnt")
            lnb = lnp.tile([1, 2 * T], bf16, tag="lnb")
            for nb in range(NB):
                tsl = slice(nb * 512, (nb + 1) * 512)
                c0 = lnt[0:1, nb * 512:(nb + 1) * 512]               # mean
                c1 = lnt[0:1, T + nb * 512: T + (nb + 1) * 512]      # var->std
                aa = lnt[0:1, 2 * T + nb * 512: 2 * T + (nb + 1) * 512]
                c0b = lnb[0:1, tsl]
                aab = lnb[0:1, T + nb * 512: T + (nb + 1) * 512]
                nc.vector.tensor_scalar_mul(c0, s1[nb][0:1, :], 2.0 / D)
                nc.scalar.square(aa, c0)
                nc.vector.scalar_tensor_tensor(c1, s2[nb][0:1, :], 4.0 / D, aa,
                                               op0=Alu.mult, op1=Alu.subtract)
                nc.scalar.activation(c1, c1, AF.Sqrt, bias=eps_t[0:1, 0:1])
                nc.vector.reciprocal_approx_fast(aa, c1)
                nc.vector.scalar_tensor_tensor(c0b, c0, -1.0, aa,
                                               op0=Alu.mult, op1=Alu.mult)
                nc.vector.tensor_scalar_mul(aab, aa, 2.0)
                pa = ps_sm.tile([P, 512], f32, tag="ps")
                nc.tensor.matmul(pa[:], ones_b[0:1, 0:P], aab,
                                 start=True, stop=True)
                nc.any.tensor_copy(abc[:, tsl], pa[:])
                pc = ps_sm.tile([P, 512], f32, tag="ps")
                nc.tensor.matmul(pc[:], ones_b[0:1, 0:P], c0b,
                                 start=True, stop=True)
                nc.any.tensor_copy(abc[:, T + nb * 512: T + (nb + 1) * 512], pc[:])
                for k in range(KD):
                    ksl = slice(k * T + nb * 512, k * T + (nb + 1) * 512)
                    nc.vector.tensor_mul(dst[:, ksl], src[:, ksl], abc[:, tsl])
                    nc.vector.tensor_add(dst[:, ksl], dst[:, ksl],
                                         abc[:, T + nb * 512: T + (nb + 1) * 512])

        for l in range(L):
            bqk_t = bp.tile([P, KD], f32, tag="bias")
            nc.sync.dma_start(out=bqk_t[:], in_=din["bqk"][l])
            bo_r = bp.tile([1, KD * P], bf16, tag="borow")
            nc.sync.dma_start(
                out=bo_r[0:1, :].rearrange("o (k c) -> o k c", k=KD),
                in_=din["bo_row"][l])
            bup_t = bp.tile([P, KFF], f32, tag="bias")
            nc.sync.dma_start(out=bup_t[:], in_=din["bup"][l])

            # ---- LN1 + qk GEMM ----
            layernorm(h, xln)
            if l == 0:
                dbg_dump("dbg_xln", xln[:])
            for m in range(KD):
                wt = wqkp.tile([P, KD * P], bf16, tag="wqk")
                nc.sync.dma_start(
                    out=wt[:].rearrange("p (k c) -> p k c", k=KD),
                    in_=din["wqk"][l, m])
                for nb in range(NB):
                    ps = ps_sm.tile([P, 512], f32, tag="ps")
                    for k in range(KD):
                        nc.tensor.matmul(
                            ps[:], wt[:, k * P:(k + 1) * P],
                            xln[:, k * T + nb * 512: k * T + (nb + 1) * 512],
                            start=(k == 0), stop=(k == KD - 1))
                    nc.scalar.activation(
                        qk[:, m * T + nb * 512: m * T + (nb + 1) * 512],
                        ps[:], AF.Identity, bias=bqk_t[:, m:m + 1])
            if l == 0:
                dbg_dump("dbg_qk", qk[:])
            # ---- v GEMM (x-stationary): vT[t, 65h+dv], col 64 = ones ----
            wv_t = wvp.tile([P, KD * DQK], bf16)
            nc.sync.dma_start(
                out=wv_t[:].rearrange("p (k c) -> p k c", k=KD),
                in_=din["wv"][l])
            for m in range(NT):
                ps = ps_sm.tile([P, 512], f32, tag="ps")
                for k in range(KD):
                    nc.tensor.matmul(
                        ps[:, 0:DQK], xln[:, k * T + m * P: k * T + (m + 1) * P],
                        wv_t[:, k * DQK:(k + 1) * DQK],
                        start=(k == 0), stop=(k == KD - 1))
                nc.any.tensor_copy(vT_4d[:, m, :, 0:64],
                                   ps[:, 0:DQK].rearrange("p (h c) -> p h c", h=NH))
            if l == 0:
                dbg_dump("dbg_vt", vT[:])

            # ---- attention: head pairs interleaved, 512-col query windows ----
            for hgrp in range(NH // 3):
                pair = (3 * hgrp, 3 * hgrp + 1, 3 * hgrp + 2)
                for qh in range(NB):
                    ktmax = 4 * (qh + 1)
                    avt = {}
                    for hh in pair:
                        avt[hh] = ps_av.tile([VW, 512], f32, tag="av",
                                             name=f"av_{hh}")
                    for kt in range(ktmax):
                        a = (kt % 4) * P if (kt // 4) == qh else 0
                        for hh in pair:
                            po = 64 * (hh % 2)
                            qc = (hh // 2) * T + qh * 512
                            kc = (3 + hh // 2) * T
                            st = ps_sm.tile([P, 512], f32, tag="ps")
                            nc.tensor.matmul(
                                st[:, a:512],
                                qk[po:po + 64, kc + kt * P: kc + (kt + 1) * P],
                                qk[po:po + 64, qc + a: qc + 512],
                                start=True, stop=((kt // 4) != qh),
                                skip_group_check=True)
                            if (kt // 4) == qh:
                                nc.tensor.matmul(st[:, a:a + P], maskT[:], ident[:],
                                                 start=False, stop=True,
                                                 skip_group_check=True)
                            pt = ptp.tile([P, 512], bf16, tag="pt")
                            nc.scalar.activation(pt[:, a:512], st[:, a:512],
                                                 AF.Exp, scale=1.0 / np.sqrt(HD))
                            nc.tensor.matmul(
                                avt[hh][0:VW, a:512],
                                vT_4d[:, kt, hh, :],
                                pt[:, a:512],
                                start=(kt == 0),
                                stop=(kt == ktmax - 1),
                                skip_group_check=True)
                    for hh in pair:
                        av = avt[hh]
                        rsum = rsp.tile([1, 512], f32, tag="rsum")
                        rs = rsp.tile([1, 512], f32, tag="rs")
                        rsb = rsp.tile([1, 512], bf16, tag="rsb")
                        nc.scalar.activation(rsum[0:1, :], av[64:65, 0:512],
                                             AF.Identity)
                        nc.vector.reciprocal_approx_fast(rs[0:1, :], rsum[0:1, :])
                        nc.gpsimd.tensor_copy(rsb[0:1, :], rs[0:1, :])
                        rb = ps_sm.tile([64, 512], f32, tag="ps")
                        nc.tensor.matmul(rb[:], ones_b[0:1, 0:64], rsb[0:1, :],
                                         start=True, stop=True,
                                         skip_group_check=True)
                        rbc = rbcp.tile([64, 512], f32, tag="rbc")
                        nc.any.tensor_copy(rbc[:], rb[:])
                        nc.vector.tensor_mul(
                            y[0:64, hh * T + qh * 512: hh * T + (qh + 1) * 512],
                            av[0:64, 0:512], rbc[:])
            if l == 0:
                dbg_dump("dbg_y", y[:])

            # ---- out_proj -> partial = h/2 + Wo/2 y + bo/2; AR in 2 halves ----
            ar_inA = dram.tile([P, 3, T], bf16, tag="ar_inA")
            ar_inB = dram.tile([P, 3, T], bf16, tag="ar_inB")
            ar_outA = dram.tile([P, 3, T], bf16, tag="ar_outA")
            ar_outB = dram.tile([P, 3, T], bf16, tag="ar_outB")
            h3 = h[:].rearrange("p (k t) -> p k t", k=KD)
            for m in range(KD):
                wt = wop.tile([64, NH * P], bf16, tag="wo")
                nc.sync.dma_start(
                    out=wt[:].rearrange("p (k c) -> p k c", k=NH),
                    in_=din["wo"][l, m])
                for nb in range(NB):
                    ps = ps_sm.tile([P, 512], f32, tag="ps")
                    for k in range(NH):
                        nc.tensor.matmul(
                            ps[:], wt[0:64, k * P:(k + 1) * P],
                            y[0:64, k * T + nb * 512: k * T + (nb + 1) * 512],
                            start=(k == 0), stop=False)
                    nc.tensor.matmul(ps[:], bo_r[0:1, m * P:(m + 1) * P],
                                     ones_b[0:1, 0:512],
                                     start=False, stop=True)
                    dst = partial[:, m * T + nb * 512: m * T + (nb + 1) * 512]
                    nc.vector.scalar_tensor_tensor(
                        dst, h[:, m * T + nb * 512: m * T + (nb + 1) * 512],
                        0.5, ps[:], op0=Alu.mult, op1=Alu.add)
                    art = ar_inA if m < 3 else ar_inB
                    nc.sync.dma_start(
                        out=art[:, m % 3, nb * 512:(nb + 1) * 512], in_=dst)
                if m == 2:
                    nc.gpsimd.collective_compute(
                        "AllReduce", Alu.add, replica_groups=groups,
                        ins=[ar_inA.opt()], outs=[ar_outA.opt()])
            nc.gpsimd.collective_compute(
                "AllReduce", Alu.add, replica_groups=groups,
                ins=[ar_inB.opt()], outs=[ar_outB.opt()])
            nc.sync.dma_start(out=h3[:, 0:3, :], in_=ar_outA[:])
            nc.sync.dma_start(out=h3[:, 3:6, :], in_=ar_outB[:])
            if l == 0:
                dbg_dump("dbg_h1", h[:])

            # ---- LN2 + MLP ----
            layernorm(h, xln)
            for m in range(KFF):
                wt = wupp.tile([P, KD * P], bf16, tag="wup")
                nc.sync.dma_start(
                    out=wt[:].rearrange("p (k c) -> p k c", k=KD),
                    in_=din["wup"][l, m])
                for nb in range(NB):
                    ps = ps_sm.tile([P, 512], f32, tag="ps")
                    for k in range(KD):
                        nc.tensor.matmul(
                            ps[:], wt[:, k * P:(k + 1) * P],
                            xln[:, k * T + nb * 512: k * T + (nb + 1) * 512],
                            start=(k == 0), stop=(k == KD - 1))
                    nc.scalar.activation(
                        g[:, m * T + nb * 512: m * T + (nb + 1) * 512],
                        ps[:], AF.Gelu, bias=bup_t[:, m:m + 1])
            ar_inA2 = dram.tile([P, 3, T], bf16, tag="ar_inA")
            ar_inB2 = dram.tile([P, 3, T], bf16, tag="ar_inB")
            ar_outA2 = dram.tile([P, 3, T], bf16, tag="ar_outA")
            ar_outB2 = dram.tile([P, 3, T], bf16, tag="ar_outB")
            for m in range(KD):
                wt = wdnp.tile([P, KFF * P], bf16, tag="wdn")
                nc.sync.dma_start(
                    out=wt[:].rearrange("p (k c) -> p k c", k=KFF),
                    in_=din["wdn"][l, m])
                for nb in range(NB):
                    ps = ps_sm.tile([P, 512], f32, tag="ps")
                    for k in range(KFF):
                        nc.tensor.matmul(
                            ps[:], wt[:, k * P:(k + 1) * P],
                            g[:, k * T + nb * 512: k * T + (nb + 1) * 512],
                            start=(k == 0), stop=(k == KFF - 1))
                    dst = partial[:, m * T + nb * 512: m * T + (nb + 1) * 512]
                    nc.vector.scalar_tensor_tensor(
                        dst, h[:, m * T + nb * 512: m * T + (nb + 1) * 512],
                        0.5, ps[:], op0=Alu.mult, op1=Alu.add)
                    art = ar_inA2 if m < 3 else ar_inB2
                    nc.sync.dma_start(
                        out=art[:, m % 3, nb * 512:(nb + 1) * 512], in_=dst)
                if m == 2:
                    nc.gpsimd.collective_compute(
                        "AllReduce", Alu.add, replica_groups=groups,
                        ins=[ar_inA2.opt()], outs=[ar_outA2.opt()])
            nc.gpsimd.collective_compute(
                "AllReduce", Alu.add, replica_groups=groups,
                ins=[ar_inB2.opt()], outs=[ar_outB2.opt()])
            h3b = h[:].rearrange("p (k t) -> p k t", k=KD)
            nc.sync.dma_start(out=h3b[:, 0:3, :], in_=ar_outA2[:])
            nc.sync.dma_start(out=h3b[:, 3:6, :], in_=ar_outB2[:])
            if l == 0:
                dbg_dump("dbg_h2", h[:])

        # ---- final LN (lnf folded into temb on host) + lm_head ----
        layernorm(h, xln)
        for n in range(VN):
            te = tep.tile([P, KD * VBLK], bf16, tag="te")
            nc.sync.dma_start(
                out=te[:].rearrange("p (k c) -> p k c", k=KD),
                in_=din["temb"][n])
            for m in range(NT):
                ps = ps_sm.tile([P, 512], f32, tag="ps")
                for k in range(KD):
                    nc.tensor.matmul(
                        ps[:, 0:VBLK], xln[:, k * T + m * P: k * T + (m + 1) * P],
                        te[:, k * VBLK:(k + 1) * VBLK],
                        start=(k == 0), stop=(k == KD - 1))
                ot = osp.tile([P, VBLK], bf16, tag="ot")
                nc.any.tensor_copy(ot[:], ps[:, 0:VBLK])
                nc.sync.dma_start(
                    out=dout[m * P:(m + 1) * P, n * VBLK:(n + 1) * VBLK],
                    in_=ot[:])
        pers.close()

    nc.compile()
    return nc


# --------------------------------------------------------------------------
# entry point
# --------------------------------------------------------------------------

def kernel(**inputs):
    import time
    t0 = time.time()
    in_maps = prep_all_inputs(**inputs)
    _CACHE["t_prep"] = time.time() - t0

    dbg = bool(int(os.environ.get("GPT_DEBUG", "0")))
    key = "nc_dbg" if dbg else "nc"
    if key not in _CACHE:
        t0 = time.time()
        _CACHE[key] = build_program(debug=dbg)
        _CACHE["t_build"] = time.time() - t0
    nc = _CACHE[key]

    from concourse.bass_utils import run_bass_kernel_spmd
    t0 = time.time()
    want_trace = bool(int(os.environ.get("GPT_TRACE", "0")))
    try:
        res = run_bass_kernel_spmd(nc, in_maps, core_ids=list(range(NC_)),
                                   trace=want_trace)
    except ModuleNotFoundError:
        res = run_bass_kernel_spmd(nc, in_maps, core_ids=list(range(NC_)),
                                   trace=False)
    _CACHE["t_run"] = time.time() - t0
    _CACHE["last_result"] = res

    logits = np.empty((B, T, V), np.float32)
    for c in range(NC_):
        b, r = c // TP, c % TP
        logits[b, :, r * VSH:(r + 1) * VSH] = np.asarray(
            res.results[c]["logits"], dtype=np.float32)

    lnf_b = np.asarray(inputs["lnf_b"], np.float32)
    if np.any(lnf_b):
        corr = np.asarray(inputs["tok_emb"], np.float32) @ lnf_b
        logits += corr[None, None, :]
    return logits


# revision 30
# speedup vs baseline: 1.1104x; 1.1104x over previous
"""GPT forward (V=32000,S=1024,D=768,L=6,H=12,FF=3072,B=4) on 8 trn2 NeuronCores.

Sharding: DP=4 core-pairs over batch B; TP=2 (Megatron) inside each pair:
  heads 6+6, FF 1536+1536, vocab 16000+16000 for the logits GEMM.
All GEMMs run in bf16 (PE 1 cycle/row vs 4 for fp32); PSUM accumulation is
fp32. Residual h is bf16. LayerNorm stats come from ones-matmul column sums
(sum and sum-of-squares), the per-token scale/shift are broadcast across
partitions with K=1 matmuls (float32r bitcast, full-rate), and applied as
two bf16 tensor_tensor ops.
Attention is transposed (sT[k,q]) and processed per (head, key-block):
one wide scores matmul per key block over all later queries, causal mask
added in PSUM via a constant-matrix matmul (-240 strictly-lower), a single
wide Exp per key block, and PV accumulated in PSUM across key blocks.
The softmax denominator rides along as a 65th ones-column of V; the final
normalize is reciprocal_approx_fast + K=1 broadcast matmul + one multiply.
TP AllReduce runs in bf16 with a Shared-scratchpad output buffer.
"""

import os
import sys

import numpy as np

for _p in ("/opt/trn_rl_repo",):
    if _p not in sys.path:
        sys.path.insert(0, _p)

V, S, D, L, H, FF = 32000, 1024, 768, 6, 12, 3072
B, T = 4, 1024
HD = D // H            # 64
NC_ = 8                # cores
TP = 2
NH = H // TP           # 6 local heads
DQK = NH * HD          # 384
FFSH = FF // TP        # 1536
VSH = V // TP          # 16000
P = 128
KD = D // P            # 6 k-chunks of d_model
KFF = FFSH // P        # 12
NT = T // P            # 8 token chunks
NB = 2                 # 512-wide token blocks
VBLK = 500             # vocab free-block
VN = VSH // VBLK       # 32
EPS = 1e-5
VW = 65                # v columns per head incl. ones column
MASKVAL = -240.0

_CACHE = {}


# --------------------------------------------------------------------------
# host-side input preparation (sharding + layout + LN folding)
# --------------------------------------------------------------------------

def _lhsT_layout(Wf, nm, nk):
    """Wf [nm*128 out, nk*128 in] -> [nm, 128(p=in%128), nk, 128(c=out%128)]
    so that sbuf tile[p, k*128+c] = Wf[m*128+c, k*128+p]."""
    return np.ascontiguousarray(
        Wf.reshape(nm, P, nk, P).transpose(0, 3, 2, 1)
    )


def _rhs_layout(Wf, nk, nblk):
    """Wf [nblk out, nk*128 in] -> [128(p), nk, nblk]: tile[p, k, c] = Wf[c, k*128+p]."""
    return np.ascontiguousarray(
        Wf.reshape(nblk, nk, P).transpose(2, 1, 0)
    )


def _bias_layout(b, nm):
    """b [nm*128] -> [128, nm]"""
    return np.ascontiguousarray(b.reshape(nm, P).T)


def prep_rank_weights(r, tok_emb, pos_emb, ln1_w, ln1_b, qkv_w, out_w,
                      ln2_w, ln2_b, up_w, down_w, lnf_w, lnf_b):
    """Weights depend only on the TP rank r (shared across the 4 DP pairs)."""
    import ml_dtypes
    bf16 = ml_dtypes.bfloat16
    f32 = np.float32

    inp = {}
    wqk = np.empty((L, KD, P, KD, P), bf16)
    bqk = np.empty((L, P, KD), f32)
    wv = np.empty((L, P, KD, DQK), bf16)
    wo = np.empty((L, KD, 64, NH, P), bf16)
    bo_row = np.empty((L, KD, P), bf16)
    wup = np.empty((L, KFF, P, KD, P), bf16)
    bup = np.empty((L, P, KFF), f32)
    wdn = np.empty((L, KD, P, KFF, P), bf16)

    hsel = slice(r * DQK, (r + 1) * DQK)
    for l in range(L):
        q_raw = qkv_w[l, 0 * D + r * DQK: 0 * D + (r + 1) * DQK]   # [384, 768]
        k_raw = qkv_w[l, 1 * D + r * DQK: 1 * D + (r + 1) * DQK]
        v_raw = qkv_w[l, 2 * D + r * DQK: 2 * D + (r + 1) * DQK]
        qk_raw = np.concatenate([q_raw, k_raw], 0)                 # [768, 768]
        wqk[l] = _lhsT_layout(qk_raw * ln1_w[l][None, :], KD, KD)
        bqk[l] = _bias_layout(qk_raw @ ln1_b[l], KD)
        wv[l] = _rhs_layout(v_raw * ln1_w[l][None, :], KD, DQK)
        bv = v_raw @ ln1_b[l]                                      # [384]
        wo_raw = out_w[l][:, hsel]                                 # [768, 384]
        # halved: the pair AllReduce of (h/2 + Wo/2 y + bo/2) returns h_new/2
        # [m, p(64), hh, c]: tile[p, hh*128+c] = wo_raw[m*128+c, hh*64+p]
        wo[l] = np.ascontiguousarray(
            (0.5 * wo_raw).reshape(KD, P, NH, 64).transpose(0, 3, 2, 1))
        bo_row[l] = (0.5 * (wo_raw @ bv)).reshape(KD, P)
        up_raw = up_w[l, r * FFSH:(r + 1) * FFSH]                  # [1536, 768]
        wup[l] = _lhsT_layout(up_raw * ln2_w[l][None, :], KFF, KD)
        bup[l] = _bias_layout(up_raw @ ln2_b[l], KFF)
        dn_raw = down_w[l][:, r * FFSH:(r + 1) * FFSH]             # [768, 1536]
        wdn[l] = _lhsT_layout(0.5 * dn_raw, KD, KFF)

    inp["wqk"], inp["bqk"], inp["wv"] = wqk, bqk, wv
    inp["wo"], inp["bo_row"] = wo, bo_row
    inp["wup"], inp["bup"], inp["wdn"] = wup, bup, wdn

    te = tok_emb[r * VSH:(r + 1) * VSH].astype(f32) * lnf_w[None, :].astype(f32)
    # [VN, 128, KD, VBLK]: tile[n, p, k, c] = te[n*VBLK + c, k*128 + p]
    inp["temb"] = np.ascontiguousarray(
        te.reshape(VN, VBLK, KD, P).transpose(0, 3, 2, 1)
    ).astype(bf16)
    return inp


def prep_all_inputs(**inputs):
    import ml_dtypes
    bf16 = ml_dtypes.bfloat16
    f32 = np.float32
    args = {k: np.asarray(v) for k, v in inputs.items()}
    for k in args:
        if args[k].dtype in (np.float64,):
            args[k] = args[k].astype(f32)
    idx = args.pop("idx")
    rank_w = [prep_rank_weights(r, **args) for r in range(TP)]

    in_maps = []
    for c in range(NC_):
        b, r = c // TP, c % TP
        inp = dict(rank_w[r])
        h0 = 0.5 * (args["tok_emb"][idx[b]] + args["pos_emb"][:T]).astype(f32).T
        inp["h0"] = np.ascontiguousarray(
            h0.reshape(KD, P, T).transpose(1, 0, 2)).astype(bf16)
        in_maps.append(inp)
    return in_maps


# --------------------------------------------------------------------------
# bass program
# --------------------------------------------------------------------------

def build_program(debug=False):
    import concourse.bass as bass
    import concourse.mybir as mybir
    import concourse.tile as tile
    from concourse import bacc
    from concourse.masks import make_upper_triangular, make_identity
    from contextlib import ExitStack

    f32 = mybir.dt.float32
    f32r = mybir.dt.float32r
    bf16 = mybir.dt.bfloat16
    AF = mybir.ActivationFunctionType
    Alu = mybir.AluOpType

    nc = bacc.Bacc(None, target_bir_lowering=False, debug=False, num_devices=NC_)

    din = {}
    din["h0"] = nc.dram_tensor("h0", [P, KD, T], bf16, kind="ExternalInput")
    din["wqk"] = nc.dram_tensor("wqk", [L, KD, P, KD, P], bf16, kind="ExternalInput")
    din["bqk"] = nc.dram_tensor("bqk", [L, P, KD], f32, kind="ExternalInput")
    din["wv"] = nc.dram_tensor("wv", [L, P, KD, DQK], bf16, kind="ExternalInput")
    din["wo"] = nc.dram_tensor("wo", [L, KD, 64, NH, P], bf16, kind="ExternalInput")
    din["bo_row"] = nc.dram_tensor("bo_row", [L, KD, P], bf16, kind="ExternalInput")
    din["wup"] = nc.dram_tensor("wup", [L, KFF, P, KD, P], bf16, kind="ExternalInput")
    din["bup"] = nc.dram_tensor("bup", [L, P, KFF], f32, kind="ExternalInput")
    din["wdn"] = nc.dram_tensor("wdn", [L, KD, P, KFF, P], bf16, kind="ExternalInput")
    din["temb"] = nc.dram_tensor("temb", [VN, P, KD, VBLK], bf16, kind="ExternalInput")
    dout = nc.dram_tensor("logits", [T, VSH], bf16, kind="ExternalOutput")
    ddbg = {}
    if debug:
        for nm, shp in (("dbg_xln", [P, KD * T]), ("dbg_qk", [P, KD * T]),
                        ("dbg_vt", [P, NT * NH * VW]),
                        ("dbg_y", [64, NH * T]),
                        ("dbg_h1", [P, KD * T]), ("dbg_h2", [P, KD * T]),
                        ("dbg_st", [P, T]), ("dbg_pt", [P, T]),
                        ("dbg_av", [VW, T])):
            ddbg[nm] = nc.dram_tensor(nm, shp, bf16, kind="ExternalOutput")
        ddbg["dbg_rs"] = nc.dram_tensor("dbg_rs", [1, T], f32, kind="ExternalOutput")
        ddbg["dbg_rbc"] = nc.dram_tensor("dbg_rbc", [64, T], f32, kind="ExternalOutput")

    groups = [[2 * i, 2 * i + 1] for i in range(NC_ // TP)]

    with tile.TileContext(nc) as tc:
        pers = ExitStack()

        const = pers.enter_context(tc.tile_pool(name="const", bufs=1))
        ones_b = const.tile([P, 512], bf16)
        nc.vector.memset(ones_b[:], 1.0)
        maskT = const.tile([P, P], bf16)
        make_upper_triangular(nc, maskT[:], val=MASKVAL, diag=False)
        ident = const.tile([P, P], bf16)
        make_identity(nc, ident[:])
        eps_t = const.tile([1, 1], f32)
        nc.vector.memset(eps_t[:], EPS)

        hp = pers.enter_context(tc.tile_pool(name="hp", bufs=1))
        h = hp.tile([P, KD * T], bf16)          # resident value is h/2
        xp = pers.enter_context(tc.tile_pool(name="xp", bufs=1))
        xln = xp.tile([P, KD * T], bf16)
        qkp = pers.enter_context(tc.tile_pool(name="qkp", bufs=1))
        qk = qkp.tile([P, KD * T], bf16)
        vtp = pers.enter_context(tc.tile_pool(name="vtp", bufs=1))
        vT = vtp.tile([P, NT * NH * VW], bf16)
        yp = pers.enter_context(tc.tile_pool(name="yp", bufs=1))
        y = yp.tile([64, NH * T], bf16)
        gp = pers.enter_context(tc.tile_pool(name="gp", bufs=1))
        g = gp.tile([P, KFF * T], bf16)
        prtp = pers.enter_context(tc.tile_pool(name="prtp", bufs=1))
        partial = prtp.tile([P, KD * T], bf16)
        abcp = pers.enter_context(tc.tile_pool(name="abcp", bufs=1))
        abc = abcp.tile([P, 2 * T], bf16)

        ptp = pers.enter_context(tc.tile_pool(name="ptp", bufs=4))
        sqp = pers.enter_context(tc.tile_pool(name="sqp", bufs=2))
        rbcp = pers.enter_context(tc.tile_pool(name="rbcp", bufs=2))
        lnp = pers.enter_context(tc.tile_pool(name="lnp", bufs=1))
        rsp = pers.enter_context(tc.tile_pool(name="rsp", bufs=2))

        # 4 + 4 PSUM banks: every tile is <= [128, 512] fp32 (one bank)
        ps_sm = pers.enter_context(tc.tile_pool(name="ps_sm", bufs=4, space="PSUM"))
        ps_av = pers.enter_context(tc.tile_pool(name="ps_av", bufs=4, space="PSUM"))

        dram = pers.enter_context(tc.tile_pool(name="dram", bufs=4, space="DRAM"))
        dbp = pers.enter_context(tc.tile_pool(name="dbp", bufs=2)) if debug else None

        wqkp = pers.enter_context(tc.tile_pool(name="wqkp", bufs=3))
        wvp = pers.enter_context(tc.tile_pool(name="wvp", bufs=1))
        wop = pers.enter_context(tc.tile_pool(name="wop", bufs=2))
        wupp = pers.enter_context(tc.tile_pool(name="wupp", bufs=3))
        wdnp = pers.enter_context(tc.tile_pool(name="wdnp", bufs=2))
        tep = pers.enter_context(tc.tile_pool(name="tep", bufs=3))
        osp = pers.enter_context(tc.tile_pool(name="osp", bufs=2))
        bp = pers.enter_context(tc.tile_pool(name="bp", bufs=3))

        nc.sync.dma_start(out=h[:].rearrange("p (k t) -> p k t", k=KD),
                          in_=din["h0"][:])
        # ones column (index 64) of every per-head V block, for denominator
        vT_4d = vT[:].rearrange("p (t h c) -> p t h c", t=NT, h=NH)
        nc.vector.memset(vT_4d[:, :, :, 64:65], 1.0)

        def dbg_dump(name, ap):
            if debug:
                nc.sync.dma_start(out=ddbg[name][:, :], in_=ap)

        def layernorm(src, dst):
            """Per-token LN of 2*src (src holds h/2) -> dst, [P, KD*T] bf16."""
            s1 = [ps_av.tile([1, 512], f32, tag="av", name=f"s1_{i}")
                  for i in range(NB)]
            s2 = [ps_av.tile([1, 512], f32, tag="av", name=f"s2_{i}")
                  for i in range(NB)]
            for k in range(KD):
                sq = sqp.tile([P, T], bf16, tag="sq")
                nc.scalar.square(sq[:], src[:, k * T:(k + 1) * T])
                for nb in range(NB):
                    nc.tensor.matmul(s1[nb][0:1, :], ones_b[:, 0:1],
                                     src[:, k * T + nb * 512: k * T + (nb + 1) * 512],
                                     start=(k == 0), stop=(k == KD - 1),
                                     skip_group_check=True)
                    nc.tensor.matmul(s2[nb][0:1, :], ones_b[:, 0:1],
                                     sq[:, nb * 512:(nb + 1) * 512],
                                     start=(k == 0), stop=(k == KD - 1),
                                     skip_group_check=True)
            lnt = lnp.tile([1, 3 * T], f32, tag="lnt")
            lnb = lnp.tile([1, 2 * T], bf16, tag="lnb")
            for nb in range(NB):
                tsl = slice(nb * 512, (nb + 1) * 512)
                c0 = lnt[0:1, nb * 512:(nb + 1) * 512]               # mean
                c1 = lnt[0:1, T + nb * 512: T + (nb + 1) * 512]      # var->std
                aa = lnt[0:1, 2 * T + nb * 512: 2 * T + (nb + 1) * 512]
                c0b = lnb[0:1, tsl]
                aab = lnb[0:1, T + nb * 512: T + (nb + 1) * 512]
                nc.vector.tensor_scalar_mul(c0, s1[nb][0:1, :], 2.0 / D)
                nc.scalar.square(aa, c0)
                nc.vector.scalar_tensor_tensor(c1, s2[nb][0:1, :], 4.0 / D, aa,
                                               op0=Alu.mult, op1=Alu.subtract)
                nc.scalar.activation(c1, c1, AF.Sqrt, bias=eps_t[0:1, 0:1])
                nc.vector.reciprocal_approx_fast(aa, c1)
                nc.vector.scalar_tensor_tensor(c0b, c0, -1.0, aa,
                                               op0=Alu.mult, op1=Alu.mult)
                nc.vector.tensor_scalar_mul(aab, aa, 2.0)
                pa = ps_sm.tile([P, 512], f32, tag="ps")
                nc.tensor.matmul(pa[:], ones_b[0:1, 0:P], aab,
                                 start=True, stop=True)
                nc.any.tensor_copy(abc[:, tsl], pa[:])
                pc = ps_sm.tile([P, 512], f32, tag="ps")
                nc.tensor.matmul(pc[:], ones_b[0:1, 0:P], c0b,
                                 start=True, stop=True)
                nc.any.tensor_copy(abc[:, T + nb * 512: T + (nb + 1) * 512], pc[:])
                for k in range(KD):
                    ksl = slice(k * T + nb * 512, k * T + (nb + 1) * 512)
                    nc.vector.tensor_mul(dst[:, ksl], src[:, ksl], abc[:, tsl])
                    nc.vector.tensor_add(dst[:, ksl], dst[:, ksl],
                                         abc[:, T + nb * 512: T + (nb + 1) * 512])

        for l in range(L):
            bqk_t = bp.tile([P, KD], f32, tag="bias")
            nc.sync.dma_start(out=bqk_t[:], in_=din["bqk"][l])
            bo_r = bp.tile([1, KD * P], bf16, tag="borow")
            nc.sync.dma_start(
                out=bo_r[0:1, :].rearrange("o (k c) -> o k c", k=KD),
                in_=din["bo_row"][l])
            bup_t = bp.tile([P, KFF], f32, tag="bias")
            nc.sync.dma_start(out=bup_t[:], in_=din["bup"][l])

            # ---- LN1 + qk GEMM ----
            layernorm(h, xln)
            if l == 0:
                dbg_dump("dbg_xln", xln[:])
            for m in range(KD):
                wt = wqkp.tile([P, KD * P], bf16, tag="wqk")
                nc.sync.dma_start(
                    out=wt[:].rearrange("p (k c) -> p k c", k=KD),
                    in_=din["wqk"][l, m])
                for nb in range(NB):
                    ps = ps_sm.tile([P, 512], f32, tag="ps")
                    for k in range(KD):
                        nc.tensor.matmul(
                            ps[:], wt[:, k * P:(k + 1) * P],
                            xln[:, k * T + nb * 512: k * T + (nb + 1) * 512],
                            start=(k == 0), stop=(k == KD - 1))
                    nc.scalar.activation(
                        qk[:, m * T + nb * 512: m * T + (nb + 1) * 512],
                        ps[:], AF.Identity, bias=bqk_t[:, m:m + 1])
            if l == 0:
                dbg_dump("dbg_qk", qk[:])
            # ---- v GEMM (x-stationary): vT[t, 65h+dv], col 64 = ones ----
            wv_t = wvp.tile([P, KD * DQK], bf16)
            nc.sync.dma_start(
                out=wv_t[:].rearrange("p (k c) -> p k c", k=KD),
                in_=din["wv"][l])
            for m in range(NT):
                ps = ps_sm.tile([P, 512], f32, tag="ps")
                for k in range(KD):
                    nc.tensor.matmul(
                        ps[:, 0:DQK], xln[:, k * T + m * P: k * T + (m + 1) * P],
                        wv_t[:, k * DQK:(k + 1) * DQK],
                        start=(k == 0), stop=(k == KD - 1))
                nc.any.tensor_copy(vT_4d[:, m, :, 0:64],
                                   ps[:, 0:DQK].rearrange("p (h c) -> p h c", h=NH))
            if l == 0:
                dbg_dump("dbg_vt", vT[:])

            # ---- attention: head pairs interleaved, 512-col query windows ----
            for hpair in range(NH // 2):
                pair = (2 * hpair, 2 * hpair + 1)
                for qh in range(NB):
                    ktmax = 4 * (qh + 1)
                    avt = {}
                    for hh in pair:
                        avt[hh] = ps_av.tile([VW, 512], f32, tag="av",
                                             name=f"av_{hh}")
                    for kt in range(ktmax):
                        a = (kt % 4) * P if (kt // 4) == qh else 0
                        for hh in pair:
                            po = 64 * (hh % 2)
                            qc = (hh // 2) * T + qh * 512
                            kc = (3 + hh // 2) * T
                            st = ps_sm.tile([P, 512], f32, tag="ps")
                            nc.tensor.matmul(
                                st[:, a:512],
                                qk[po:po + 64, kc + kt * P: kc + (kt + 1) * P],
                                qk[po:po + 64, qc + a: qc + 512],
                                start=True, stop=((kt // 4) != qh),
                                skip_group_check=True)
                            if (kt // 4) == qh:
                                nc.tensor.matmul(st[:, a:a + P], maskT[:], ident[:],
                                                 start=False, stop=True,
                                                 skip_group_check=True)
                            pt = ptp.tile([P, 512], bf16, tag="pt")
                            nc.scalar.activation(pt[:, a:512], st[:, a:512],
                                                 AF.Exp, scale=1.0 / np.sqrt(HD))
                            nc.tensor.matmul(
                                avt[hh][0:VW, a:512],
                                vT_4d[:, kt, hh, :],
                                pt[:, a:512],
                                start=(kt == 0),
                                stop=(kt == ktmax - 1),
                                skip_group_check=True)
                    for hh in pair:
                        av = avt[hh]
                        rsum = rsp.tile([1, 512], f32, tag="rsum")
                        rs = rsp.tile([1, 512], f32, tag="rs")
                        rsb = rsp.tile([1, 512], bf16, tag="rsb")
                        nc.scalar.activation(rsum[0:1, :], av[64:65, 0:512],
                                             AF.Identity)
                        nc.vector.reciprocal_approx_fast(rs[0:1, :], rsum[0:1, :])
                        nc.gpsimd.tensor_copy(rsb[0:1, :], rs[0:1, :])
                        rb = ps_av.tile([64, 512], f32, tag="av", name="rb")
                        nc.tensor.matmul(rb[:], ones_b[0:1, 0:64], rsb[0:1, :],
                                         start=True, stop=True,
                                         skip_group_check=True)
                        rbc = rbcp.tile([64, 512], f32, tag="rbc")
                        nc.any.tensor_copy(rbc[:], rb[:])
                        nc.vector.tensor_mul(
                            y[0:64, hh * T + qh * 512: hh * T + (qh + 1) * 512],
                            av[0:64, 0:512], rbc[:])
            if l == 0:
                dbg_dump("dbg_y", y[:])

            # ---- out_proj -> partial = h/2 + Wo/2 y + bo/2; AR in 2 halves ----
            ar_inA = dram.tile([P, 3, T], bf16, tag="ar_inA")
            ar_inB = dram.tile([P, 3, T], bf16, tag="ar_inB")
            ar_outA = dram.tile([P, 3, T], bf16, tag="ar_outA")
            ar_outB = dram.tile([P, 3, T], bf16, tag="ar_outB")
            h3 = h[:].rearrange("p (k t) -> p k t", k=KD)
            for m in range(KD):
                wt = wop.tile([64, NH * P], bf16, tag="wo")
                nc.sync.dma_start(
                    out=wt[:].rearrange("p (k c) -> p k c", k=NH),
                    in_=din["wo"][l, m])
                for nb in range(NB):
                    ps = ps_sm.tile([P, 512], f32, tag="ps")
                    for k in range(NH):
                        nc.tensor.matmul(
                            ps[:], wt[0:64, k * P:(k + 1) * P],
                            y[0:64, k * T + nb * 512: k * T + (nb + 1) * 512],
                            start=(k == 0), stop=False)
                    nc.tensor.matmul(ps[:], bo_r[0:1, m * P:(m + 1) * P],
                                     ones_b[0:1, 0:512],
                                     start=False, stop=True)
                    dst = partial[:, m * T + nb * 512: m * T + (nb + 1) * 512]
                    nc.vector.scalar_tensor_tensor(
                        dst, h[:, m * T + nb * 512: m * T + (nb + 1) * 512],
                        0.5, ps[:], op0=Alu.mult, op1=Alu.add)
                    art = ar_inA if m < 3 else ar_inB
                    nc.sync.dma_start(
                        out=art[:, m % 3, nb * 512:(nb + 1) * 512], in_=dst)
                if m == 2:
                    nc.gpsimd.collective_compute(
                        "AllReduce", Alu.add, replica_groups=groups,
                        ins=[ar_inA.opt()], outs=[ar_outA.opt()])
            nc.gpsimd.collective_compute(
                "AllReduce", Alu.add, replica_groups=groups,
                ins=[ar_inB.opt()], outs=[ar_outB.opt()])
            for kc_ in range(3):
                nc.sync.dma_start(out=h3[:, kc_:kc_ + 1, :],
                                  in_=ar_outA[:, kc_:kc_ + 1, :])
            for kc_ in range(3):
                nc.sync.dma_start(out=h3[:, 3 + kc_:4 + kc_, :],
                                  in_=ar_outB[:, kc_:kc_ + 1, :])
            if l == 0:
                dbg_dump("dbg_h1", h[:])

            # ---- LN2 + MLP ----
            layernorm(h, xln)
            for m in range(KFF):
                wt = wupp.tile([P, KD * P], bf16, tag="wup")
                nc.sync.dma_start(
                    out=wt[:].rearrange("p (k c) -> p k c", k=KD),
                    in_=din["wup"][l, m])
                for nb in range(NB):
                    ps = ps_sm.tile([P, 512], f32, tag="ps")
                    for k in range(KD):
                        nc.tensor.matmul(
                            ps[:], wt[:, k * P:(k + 1) * P],
                            xln[:, k * T + nb * 512: k * T + (nb + 1) * 512],
                            start=(k == 0), stop=(k == KD - 1))
                    nc.scalar.activation(
                        g[:, m * T + nb * 512: m * T + (nb + 1) * 512],
                        ps[:], AF.Gelu, bias=bup_t[:, m:m + 1])
            ar_inA2 = dram.tile([P, 3, T], bf16, tag="ar_inA")
            ar_inB2 = dram.tile([P, 3, T], bf16, tag="ar_inB")
            ar_outA2 = dram.tile([P, 3, T], bf16, tag="ar_outA")
            ar_outB2 = dram.tile([P, 3, T], bf16, tag="ar_outB")
            for m in range(KD):
                wt = wdnp.tile([P, KFF * P], bf16, tag="wdn")
                nc.sync.dma_start(
                    out=wt[:].rearrange("p (k c) -> p k c", k=KFF),
                    in_=din["wdn"][l, m])
                for nb in range(NB):
                    ps = ps_sm.tile([P, 512], f32, tag="ps")
                    for k in range(KFF):
                        nc.tensor.matmul(
                            ps[:], wt[:, k * P:(k + 1) * P],
                            g[:, k * T + nb * 512: k * T + (nb + 1) * 512],
                            start=(k == 0), stop=(k == KFF - 1))
                    dst = partial[:, m * T + nb * 512: m * T + (nb + 1) * 512]
                    nc.vector.scalar_tensor_tensor(
                        dst, h[:, m * T + nb * 512: m * T + (nb + 1) * 512],
                        0.5, ps[:], op0=Alu.mult, op1=Alu.add)
                    art = ar_inA2 if m < 3 else ar_inB2
                    nc.sync.dma_start(
                        out=art[:, m % 3, nb * 512:(nb + 1) * 512], in_=dst)
                if m == 2:
                    nc.gpsimd.collective_compute(
                        "AllReduce", Alu.add, replica_groups=groups,
                        ins=[ar_inA2.opt()], outs=[ar_outA2.opt()])
            nc.gpsimd.collective_compute(
                "AllReduce", Alu.add, replica_groups=groups,
                ins=[ar_inB2.opt()], outs=[ar_outB2.opt()])
            h3b = h[:].rearrange("p (k t) -> p k t", k=KD)
            for kc_ in range(3):
                nc.sync.dma_start(out=h3b[:, kc_:kc_ + 1, :],
                                  in_=ar_outA2[:, kc_:kc_ + 1, :])
            for kc_ in range(3):
                nc.sync.dma_start(out=h3b[:, 3 + kc_:4 + kc_, :],
                                  in_=ar_outB2[:, kc_:kc_ + 1, :])
            if l == 0:
                dbg_dump("dbg_h2", h[:])

        # ---- final LN (lnf folded into temb on host) + lm_head ----
        layernorm(h, xln)
        for n in range(VN):
            te = tep.tile([P, KD * VBLK], bf16, tag="te")
            nc.sync.dma_start(
                out=te[:].rearrange("p (k c) -> p k c", k=KD),
                in_=din["temb"][n])
            for m in range(NT):
                ps = ps_sm.tile([P, 512], f32, tag="ps")
                for k in range(KD):
                    nc.tensor.matmul(
                        ps[:, 0:VBLK], xln[:, k * T + m * P: k * T + (m + 1) * P],
                        te[:, k * VBLK:(k + 1) * VBLK],
                        start=(k == 0), stop=(k == KD - 1))
                ot = osp.tile([P, VBLK], bf16, tag="ot")
                nc.any.tensor_copy(ot[:], ps[:, 0:VBLK])
                nc.sync.dma_start(
                    out=dout[m * P:(m + 1) * P, n * VBLK:(n + 1) * VBLK],
                    in_=ot[:])
        pers.close()

    nc.compile()
    return nc


# --------------------------------------------------------------------------
# entry point
# --------------------------------------------------------------------------

def kernel(**inputs):
    import time
    t0 = time.time()
    in_maps = prep_all_inputs(**inputs)
    _CACHE["t_prep"] = time.time() - t0

    dbg = bool(int(os.environ.get("GPT_DEBUG", "0")))
    key = "nc_dbg" if dbg else "nc"
    if key not in _CACHE:
        t0 = time.time()
        _CACHE[key] = build_program(debug=dbg)
        _CACHE["t_build"] = time.time() - t0
    nc = _CACHE[key]

    from concourse.bass_utils import run_bass_kernel_spmd
    t0 = time.time()
    want_trace = bool(int(os.environ.get("GPT_TRACE", "0")))
    try:
        res = run_bass_kernel_spmd(nc, in_maps, core_ids=list(range(NC_)),
                                   trace=want_trace)
    except ModuleNotFoundError:
        res = run_bass_kernel_spmd(nc, in_maps, core_ids=list(range(NC_)),
                                   trace=False)
    _CACHE["t_run"] = time.time() - t0
    _CACHE["last_result"] = res

    logits = np.empty((B, T, V), np.float32)
    for c in range(NC_):
        b, r = c // TP, c % TP
        logits[b, :, r * VSH:(r + 1) * VSH] = np.asarray(
            res.results[c]["logits"], dtype=np.float32)

    lnf_b = np.asarray(inputs["lnf_b"], np.float32)
    if np.any(lnf_b):
        corr = np.asarray(inputs["tok_emb"], np.float32) @ lnf_b
        logits += corr[None, None, :]
    return logits
